# revision 13
# baseline (speedup 1.0000x reference)
"""TAGConv GNN classifier on 8 Trainium2 NeuronCores.

Sharding: nodes split into 8 contiguous slices (6250/core, padded to 6272);
edges live on the core that owns their dst. Each hop: every core gathers
src rows (fp16, 256B) from a replicated norm-prescaled node table in HBM
(dma_gather, int16 indices -> split-table trick), segment-sums them into its
dst slice with one-hot matmuls on TensorE (fp32 PSUM accumulation), rescales
by norm, and all-gathers its slice of the next table. Readout partial sums
per graph are all-reduced, then every core computes the (identical) logits.

Wall-clock optimizations vs v0: inputs shipped compact (x as int8 with
per-node dequant scales folded with the degree norm, gather indices shipped
16-partition and replicated on device, slots int8, weights fp16, norm
computed on host), the whole device pipeline runs fp16 (fp32 accumulate),
and the jitted PJRT callable is cached so warm invocations skip retrace/
recompile/NEFF reload.
"""
import os

import numpy as np

import concourse.bass as bass
import concourse.bacc as bacc
import concourse.mybir as mybir
import concourse.tile as tile

N, E, G = 50000, 800000, 128
F = 128                      # IN_DIM == HID
CLASSES = 10
HOPS, HLAYERS = 2, 2         # 3 TAGConv layers total
NCORES = 8

PER = N // NCORES            # real nodes per core
GRP = (PER + 127) // 128     # dst groups of 128 per core
NPAD = GRP * 128             # padded nodes per core
NT = NCORES * NPAD           # padded total
HALF = NT // 2               # int16-safe split of the node table

FP = mybir.dt.float32
F16 = mybir.dt.float16
I16 = mybir.dt.int16
I8 = mybir.dt.int8


def _prep_edges(src, dst):
    """Per-core gather-index + one-hot-slot tables, SPMD-uniform shapes."""
    src = src.astype(np.int64)
    dst = dst.astype(np.int64)
    core = dst // PER
    local = dst - core * PER
    grp = local // 128
    slot = local % 128
    ps = (src // PER) * NPAD + (src % PER)          # padded global src id
    half = (ps >= HALF).astype(np.int64)
    idxv = ps - half * HALF                          # int16-safe index

    key = (core * GRP + grp) * 2 + half
    order = np.argsort(key, kind="stable")
    cnt = np.bincount(key, minlength=NCORES * GRP * 2).reshape(NCORES, GRP, 2)
    CA = np.maximum(1, -(-cnt[:, :, 0].max(axis=0) // 128)).astype(int)
    CB = np.maximum(1, -(-cnt[:, :, 1].max(axis=0) // 128)).astype(int)
    nch = CA + CB                                    # chunks per group
    choff = np.concatenate([[0], np.cumsum(nch)]).astype(int)
    NCH = int(choff[-1])
    TOT = NCH * 128

    idx16 = np.zeros((NCORES, TOT), np.int16)
    slotv = np.full((NCORES, TOT), -1, np.int8)
    sidx = idxv[order]
    sslot = slot[order]
    starts = np.concatenate([[0], np.cumsum(cnt.reshape(-1))]).astype(int)
    for c in range(NCORES):
        for g in range(GRP):
            base = choff[g] * 128
            for h, off in ((0, base), (1, base + CA[g] * 128)):
                k = (c * GRP + g) * 2 + h
                n = int(cnt[c, g, h])
                s0 = starts[k]
                idx16[c, off : off + n] = sidx[s0 : s0 + n]
                slotv[c, off : off + n] = sslot[s0 : s0 + n]

    idx_w = np.stack([idx16[c].reshape(-1, 16).T for c in range(NCORES)])   # [8,16,W16]
    slot_cols = np.stack([slotv[c].reshape(NCH, 128).T for c in range(NCORES)])
    # degree norm (in-degrees over dst, clamped to 1)
    deg = np.bincount(dst, minlength=N).astype(np.float64)
    norm = np.where(deg < 1.0, 1.0, deg) ** -0.5
    return idx_w, slot_cols, norm.astype(np.float32), CA, CB, choff, NCH, TOT


def _build_program(CA, CB, choff, NCH, TOT):
    STAGE = os.environ.get("KSTAGE", "full")
    ORDER = ["t0", "ag0", "hop1", "aghop", "hop2", "layer0", "full"]
    LVL = ORDER.index(STAGE)
    nc = bacc.Bacc("TRN2", target_bir_lowering=False, debug=False, num_devices=NCORES)
    RG = [list(range(NCORES))]
    W16 = TOT // 16
    MAXCH = int(max(CA + CB))
    SHARED = os.environ.get("KSHARED", "1") == "1"
    SPKT = os.environ.get("KSPKT", "0") == "1"

    WROWS = (HLAYERS + 1) * (HOPS + 1) * F + F          # packed weight rows
    WSH = WROWS // NCORES
    XW = (F + 2) // 3 + ((F + 2) // 3 == 0)              # 43 packed uint16/node
    xp_d = nc.dram_tensor("xp", [NPAD, XW], I16, kind="ExternalInput")
    xs_d = nc.dram_tensor("xs_cols", [128, GRP], F16, kind="ExternalInput")
    normc_d = nc.dram_tensor("norm_cols", [128, GRP], F16, kind="ExternalInput")
    idx_d = nc.dram_tensor("idx_w", [16, W16], I16, kind="ExternalInput")
    slot_d = nc.dram_tensor("slot_cols", [128, NCH], I8, kind="ExternalInput")
    gslot_d = nc.dram_tensor("gslot", [128, GRP], I8, kind="ExternalInput")
    wsh_d = nc.dram_tensor("wsh", [WSH, F], F16, kind="ExternalInput")
    b_d = nc.dram_tensor("b_cols", [128, HLAYERS + 1], FP, kind="ExternalInput")
    bcr_d = nc.dram_tensor("bc_rep", [128, CLASSES], FP, kind="ExternalInput")
    out_d = nc.dram_tensor("out", [G, CLASSES], FP, kind="ExternalOutput")

    with tile.TileContext(nc) as tc:
        with (
            tc.tile_pool(name="const", bufs=1) as cp,
            tc.tile_pool(name="work", bufs=2) as wp,
            tc.tile_pool(name="psmm", bufs=3, space="PSUM") as pmm,
            tc.tile_pool(name="pstr", bufs=2, space="PSUM") as ptr,
            tc.tile_pool(name="psro", bufs=2, space="PSUM") as pro,
            tc.tile_pool(name="dram", bufs=1, space="DRAM") as dp,
        ):
            # ---- persistent tiles ----
            idx_t = cp.tile([128, W16], I16)
            slot8_t = cp.tile([128, NCH], I8)
            slot_t = cp.tile([128, NCH], F16)
            gslot8_t = cp.tile([128, GRP], I8)
            gslot_t = cp.tile([128, GRP], F16)
            valid_t = cp.tile([128, GRP], F16)
            iota_t = cp.tile([128, 128], FP)
            iota16_t = cp.tile([128, 128], F16)
            ident_t = cp.tile([128, 128], F16)
            normc_t = cp.tile([128, GRP], FP)
            normb_t = cp.tile([128, GRP], F16)
            xs_t = cp.tile([128, GRP], FP)
            xsb_t = cp.tile([128, GRP], FP)
            w_t = [cp.tile([128, HOPS + 1, F], F16, name=f"w{l}_t", tag=f"w{l}")
                   for l in range(HLAYERS + 1)]
            b_t = cp.tile([128, HLAYERS + 1], FP)
            wc_t = cp.tile([F, CLASSES], F16)
            bcr_t = cp.tile([128, CLASSES], FP)
            f0T = cp.tile([128, GRP * 128], F16)   # feat-major [f, i] per group
            f1T = cp.tile([128, GRP * 128], F16)
            f2T = cp.tile([128, GRP * 128], F16)
            roacc_t = cp.tile([128, F + 1], FP)
            ro2_t = cp.tile([128, F + 1], FP)
            cnt_t = cp.tile([128, 1], FP)
            rcp_t = cp.tile([128, 1], FP)
            hg_t = cp.tile([128, F], F16)
            hgT_t = cp.tile([F, 128], F16)
            logit_t = cp.tile([128, CLASSES], FP)

            ASP = "Shared" if SHARED else "Local"
            T_ins = [dp.tile([NT, F], F16, addr_space=ASP, name=f"T_in{l}")
                     for l in range(HLAYERS + 1)]
            T_hops = [dp.tile([NT, F], F16, addr_space=ASP, name=f"T_hop{l}")
                      for l in range(HLAYERS + 1)]
            ag_in = dp.tile([NPAD, F], F16)
            w_shi = dp.tile([WSH, F], F16)
            w_all = dp.tile([WROWS, F], F16, addr_space=ASP)
            ar_in = dp.tile([128, F + 1], FP)
            ar_out = dp.tile([128, F + 1], FP,
                             addr_space="Shared" if SHARED else "Local")

            # ---- constants ----
            for k in range(8):
                nc.sync.dma_start(idx_t[16 * k : 16 * (k + 1), :], idx_d[:, :])
            nc.sync.dma_start(slot8_t[:], slot_d[:, :])
            nc.sync.dma_start(gslot8_t[:], gslot_d[:, :])
            normh_t = cp.tile([128, GRP], F16)
            xsh_t = cp.tile([128, GRP], F16)
            nc.sync.dma_start(normh_t[:], normc_d[:, :])
            nc.sync.dma_start(xsh_t[:], xs_d[:, :])
            nc.vector.tensor_copy(normc_t[:], normh_t[:])
            nc.vector.tensor_copy(xs_t[:], xsh_t[:])
            nc.sync.dma_start(w_shi[:, :], wsh_d[:, :])
            nc.gpsimd.collective_compute(
                "AllGather", mybir.AluOpType.bypass, replica_groups=RG,
                ins=[w_shi.opt()], outs=[w_all.opt()])
            for l in range(HLAYERS + 1):
                for k in range(HOPS + 1):
                    r0 = (l * (HOPS + 1) + k) * F
                    nc.sync.dma_start(w_t[l][:, k, :], w_all[r0 : r0 + F, :])
            nc.sync.dma_start(wc_t[:], w_all[(HLAYERS + 1) * (HOPS + 1) * F :, 0:CLASSES])
            nc.sync.dma_start(b_t[:], b_d[:, :])
            nc.sync.dma_start(bcr_t[:], bcr_d[:, :])

            nc.gpsimd.iota(iota_t[:], pattern=[[1, 128]], base=0, channel_multiplier=0,
                           allow_small_or_imprecise_dtypes=True)
            icol_t = cp.tile([128, 1], FP)
            nc.gpsimd.iota(icol_t[:], pattern=[[0, 1]], base=0, channel_multiplier=1,
                           allow_small_or_imprecise_dtypes=True)
            nc.vector.tensor_copy(iota16_t[:], iota_t[:])
            icol16_t = cp.tile([128, 1], F16)
            nc.vector.tensor_copy(icol16_t[:], icol_t[:])
            nc.vector.tensor_tensor(ident_t[:], icol16_t[:].broadcast_to([128, 128]),
                                    iota16_t[:], mybir.AluOpType.is_equal)
            nc.vector.tensor_copy(slot_t[:], slot8_t[:])
            nc.vector.tensor_copy(gslot_t[:], gslot8_t[:])
            nc.vector.tensor_scalar(valid_t[:], gslot_t[:], -1.0, None,
                                    mybir.AluOpType.not_equal)
            nc.vector.tensor_copy(normb_t[:], normh_t[:])
            nc.vector.tensor_scalar(xsb_t[:], xs_t[:], -15.5, None,
                                    mybir.AluOpType.mult)
            nc.vector.memset(roacc_t[:], 0.0)

            def bail():
                nc.vector.tensor_copy(logit_t[:], iota_t[:, :CLASSES])
                nc.sync.dma_start(out_d[:, :], logit_t[:])

            def onehot_all(g):
                """[128e, nch, 128j] one-hot tile for group g (one DVE op)."""
                nch = int(CA[g] + CB[g])
                c0 = int(choff[g])
                oh = wp.tile([128, MAXCH, 128], F16, name="oh", tag="oh")
                nc.vector.tensor_tensor(
                    oh[:, :nch, :],
                    slot_t[:, c0 : c0 + nch].unsqueeze(2).broadcast_to([128, nch, 128]),
                    iota16_t[:].unsqueeze(1).broadcast_to([128, nch, 128]),
                    mybir.AluOpType.is_equal,
                )
                return oh, nch

            STOP = False

            # ---- unpack int5 x; f0 = dequant; T0 = f0*norm ; f0T = f0^T ----
            for g in range(GRP):
                gs = slice(g * 128, (g + 1) * 128)
                xq = wp.tile([128, XW], I16, name="xq", tag="xq")
                nc.sync.dma_start(xq[:], xp_d[gs, :])
                vq = wp.tile([128, 3, XW], I16, name="vq", tag="vq")
                nc.vector.tensor_scalar(vq[:, 0, :], xq[:], 31, None,
                                        mybir.AluOpType.bitwise_and)
                nc.vector.tensor_scalar(vq[:, 1, :], xq[:], 5, 31,
                                        mybir.AluOpType.logical_shift_right,
                                        mybir.AluOpType.bitwise_and)
                nc.vector.tensor_scalar(vq[:, 2, :], xq[:], 10, 31,
                                        mybir.AluOpType.logical_shift_right,
                                        mybir.AluOpType.bitwise_and)
                xg = wp.tile([128, XW, 3], F16, name="xg", tag="xg")
                for j in range(3):
                    nc.scalar.activation(xg[:, :, j : j + 1], vq[:, j, :].unsqueeze(2),
                                         mybir.ActivationFunctionType.Identity,
                                         scale=xs_t[:, g : g + 1],
                                         bias=xsb_t[:, g : g + 1])
                f0 = xg[:, :, :].rearrange("p a b -> p (a b)")[:, 0:F]
                t0 = wp.tile([128, F], F16, name="t0", tag="tn")
                nc.vector.tensor_tensor(t0[:], f0,
                                        normb_t[:, g : g + 1].broadcast_to([128, F]),
                                        mybir.AluOpType.mult)
                nc.sync.dma_start(ag_in[gs, :], t0[:])
                pt = ptr.tile([128, 128], F16, name="pt", tag="tr")
                nc.tensor.transpose(pt[:], f0, ident_t[:])
                nc.vector.tensor_copy(f0T[:, gs], pt[:])
            if LVL <= ORDER.index("t0"):
                bail()
                STOP = True
            if not STOP:
                nc.gpsimd.collective_compute(
                    "AllGather", mybir.AluOpType.bypass, replica_groups=RG,
                    ins=[ag_in.opt()], outs=[T_ins[0].opt()])
            if not STOP and LVL <= ORDER.index("ag0"):
                bail()
                STOP = True

            def hop(src_tbl, fT, make_table):
                """One SpMM hop: gather -> one-hot segsum -> scale; optionally
                also emit next scaled table slice into ag_in."""
                for g in range(GRP):
                    gs = slice(g * 128, (g + 1) * 128)
                    ca, cb = int(CA[g]), int(CB[g])
                    nch = ca + cb
                    c0 = int(choff[g])
                    vb = wp.tile([128, MAXCH, 128], F16, name="vb", tag="vb")
                    colA = c0 * 8
                    colB = colA + ca * 8
                    nc.gpsimd.dma_gather(
                        vb[:, 0:ca, :], src_tbl[:, :], idx_t[:, colA : colA + ca * 8],
                        ca * 128, ca * 128, F, single_packet=SPKT)
                    nc.gpsimd.dma_gather(
                        vb[:, ca:nch, :], src_tbl[HALF:, :], idx_t[:, colB : colB + cb * 8],
                        cb * 128, cb * 128, F, single_packet=SPKT)
                    oh, _ = onehot_all(g)
                    ps = pmm.tile([128, 128], FP, name="ps", tag="mm")
                    for c in range(nch):
                        nc.tensor.matmul(ps[:], oh[:, c, :], vb[:, c, :],
                                         start=(c == 0), stop=(c == nch - 1))
                    fn = wp.tile([128, F], F16, name="fn", tag="fn")
                    nc.vector.tensor_tensor(fn[:], ps[:],
                                            normc_t[:, g : g + 1].broadcast_to([128, F]),
                                            mybir.AluOpType.mult)
                    if make_table:
                        tn = wp.tile([128, F], F16, name="tn", tag="tn")
                        nc.vector.tensor_tensor(tn[:], fn[:],
                                                normb_t[:, g : g + 1].broadcast_to([128, F]),
                                                mybir.AluOpType.mult)
                        nc.sync.dma_start(ag_in[gs, :], tn[:])
                    pt = ptr.tile([128, 128], F16, name="pt2", tag="tr")
                    nc.tensor.transpose(pt[:], fn[:], ident_t[:])
                    nc.vector.tensor_copy(fT[:, gs], pt[:])

            for l in range(HLAYERS + 1) if not STOP else []:
                hop(T_ins[l], f1T, make_table=True)
                if l == 0 and LVL <= ORDER.index("hop1"):
                    bail()
                    STOP = True
                    break
                nc.gpsimd.collective_compute(
                    "AllGather", mybir.AluOpType.bypass, replica_groups=RG,
                    ins=[ag_in.opt()], outs=[T_hops[l].opt()])
                if l == 0 and LVL <= ORDER.index("aghop"):
                    bail()
                    STOP = True
                    break
                hop(T_hops[l], f2T, make_table=False)
                if l == 0 and LVL <= ORDER.index("hop2"):
                    bail()
                    STOP = True
                    break
                fTs = [f0T, f1T, f2T]
                for g in range(GRP):
                    gs = slice(g * 128, (g + 1) * 128)
                    ph = pmm.tile([128, 128], FP, name="ph", tag="mm")
                    for k in range(HOPS + 1):
                        nc.tensor.matmul(ph[:], w_t[l][:, k, :], fTs[k][:, gs],
                                         start=(k == 0), stop=(k == HOPS))
                    nc.scalar.activation(f0T[:, gs], ph[:],
                                         mybir.ActivationFunctionType.Relu,
                                         bias=b_t[:, l : l + 1])
                    pt = ptr.tile([128, 128], F16, name="pt3", tag="tr")
                    nc.tensor.transpose(pt[:], f0T[:, gs], ident_t[:])
                    if l < HLAYERS:
                        tn = wp.tile([128, F], F16, name="tn2", tag="tn")
                        nc.vector.tensor_tensor(tn[:], pt[:],
                                                normb_t[:, g : g + 1].broadcast_to([128, F]),
                                                mybir.AluOpType.mult)
                        nc.sync.dma_start(ag_in[gs, :], tn[:])
                    else:
                        rr = wp.tile([128, F + 1], F16, name="rr", tag="rr")
                        nc.vector.tensor_copy(rr[:, 0:F], pt[:])
                        nc.vector.tensor_copy(rr[:, F : F + 1], valid_t[:, g : g + 1])
                        og = wp.tile([128, 128], F16, name="og", tag="og")
                        nc.vector.tensor_tensor(
                            og[:], gslot_t[:, g : g + 1].broadcast_to([128, 128]),
                            iota16_t[:], mybir.AluOpType.is_equal)
                        pr = pro.tile([128, F + 1], FP, name="pr", tag="ro")
                        nc.tensor.matmul(pr[:], og[:], rr[:], start=True, stop=True)
                        nc.vector.tensor_tensor(roacc_t[:], roacc_t[:], pr[:],
                                                mybir.AluOpType.add)
                if l < HLAYERS:
                    nc.gpsimd.collective_compute(
                        "AllGather", mybir.AluOpType.bypass, replica_groups=RG,
                        ins=[ag_in.opt()], outs=[T_ins[l + 1].opt()])
                if l == 0 and LVL <= ORDER.index("layer0"):
                    bail()
                    STOP = True
                    break

            # ---- readout: all-reduce partial sums, mean, classify ----
            if not STOP:
                nc.sync.dma_start(ar_in[:, :], roacc_t[:])
                nc.gpsimd.collective_compute(
                    "AllReduce", mybir.AluOpType.add, replica_groups=RG,
                    ins=[ar_in.opt()], outs=[ar_out.opt()])
                nc.sync.dma_start(ro2_t[:], ar_out[:, :])
                nc.vector.tensor_scalar_max(cnt_t[:], ro2_t[:, F : F + 1], 1.0)
                nc.vector.reciprocal(rcp_t[:], cnt_t[:])
                nc.vector.tensor_tensor(hg_t[:], ro2_t[:, 0:F],
                                        rcp_t[:].broadcast_to([128, F]),
                                        mybir.AluOpType.mult)
                ptf = ptr.tile([128, 128], F16, name="ptf", tag="tr")
                nc.tensor.transpose(ptf[:], hg_t[:], ident_t[:])
                nc.vector.tensor_copy(hgT_t[:], ptf[:])
                plog = pro.tile([128, F + 1], FP, name="plog", tag="ro")
                nc.tensor.matmul(plog[:, 0:CLASSES], hgT_t[:], wc_t[:], start=True, stop=True)
                nc.vector.tensor_tensor(logit_t[:], plog[:, 0:CLASSES], bcr_t[:],
                                        mybir.AluOpType.add)
                nc.sync.dma_start(out_d[:, :], logit_t[:])

    nc.finalize()
    return nc


def _make_runner(nc, n_cores):
    """Build a reusable jitted callable for `nc` (skips per-call retrace +
    recompile + NEFF reload that run_bass_kernel_spmd pays)."""
    import jax
    from jax.sharding import Mesh, PartitionSpec
    from jax.experimental.shard_map import shard_map
    from concourse.bass2jax import (
        _bass_exec_p, install_neuronx_cc_hook, partition_id_tensor)

    install_neuronx_cc_hook()
    partition_name = nc.partition_id_tensor.name if nc.partition_id_tensor else None
    in_names, out_names, out_avals, zero_outs = [], [], [], []
    for alloc in nc.m.functions[0].allocations:
        if not isinstance(alloc, mybir.MemoryLocationSet):
            continue
        name = alloc.memorylocations[0].name
        if alloc.kind == "ExternalInput":
            if name != partition_name:
                in_names.append(name)
        elif alloc.kind == "ExternalOutput":
            shape = tuple(alloc.tensor_shape)
            dtype = mybir.dt.np(alloc.dtype)
            out_avals.append(jax.core.ShapedArray(shape, dtype))
            out_names.append(name)
            zero_outs.append(np.zeros(shape, dtype))
    n_params = len(in_names)
    n_outs = len(out_avals)
    in_names_all = in_names + out_names + ([partition_name] if partition_name else [])
    donate = tuple(range(n_params, n_params + n_outs))

    def _body(*args):
        operands = list(args)
        if partition_name is not None:
            operands.append(partition_id_tensor())
        outs = _bass_exec_p.bind(
            *operands,
            out_avals=tuple(out_avals),
            in_names=tuple(in_names_all),
            out_names=tuple(out_names),
            lowering_input_output_aliases=(),
            sim_require_finite=True,
            sim_require_nnan=True,
            nc=nc,
        )
        return tuple(outs)

    devices = jax.devices()[:n_cores]
    assert len(devices) >= n_cores or len(devices) == n_cores
    mesh = Mesh(np.asarray(devices), ("core",))
    in_specs = (PartitionSpec("core"),) * (n_params + n_outs)
    out_specs = (PartitionSpec("core"),) * len(out_names)
    sharded = jax.jit(
        shard_map(_body, mesh=mesh, in_specs=in_specs, out_specs=out_specs,
                  check_rep=False),
        donate_argnums=donate, keep_unused=True)

    def run(in_maps):
        import time as _time
        timing = os.environ.get("KTIME", "0") == "1"
        tA = _time.time()
        per_core = [[np.asarray(m[nm]) for nm in in_names] for m in in_maps]
        concat_in = [np.concatenate([per_core[c][i] for c in range(n_cores)], axis=0)
                     for i in range(n_params)]
        concat_zeros = [np.zeros((n_cores * z.shape[0], *z.shape[1:]), z.dtype)
                        for z in zero_outs]
        tB = _time.time()
        out_arrs = sharded(*concat_in, *concat_zeros)
        tC = _time.time()
        # fetch only core 0's shard of each output: every core computes the
        # same logits, and pulling all 8 shards costs ~100ms of serial RTTs
        res = {nm: np.asarray(out_arrs[i].addressable_shards[0].data)
               for i, nm in enumerate(out_names)}
        tD = _time.time()
        if timing:
            print(f"  concat {1e3*(tB-tA):.0f} dispatch {1e3*(tC-tB):.0f} "
                  f"exec+fetch {1e3*(tD-tC):.0f} ms", flush=True)
        return [res]

    return run


_RUNNER_CACHE: dict = {}


def _get_runner(sig, CA, CB, choff, NCH, TOT):
    if sig not in _RUNNER_CACHE:
        nc = _build_program(CA, CB, choff, NCH, TOT)
        _RUNNER_CACHE[sig] = _make_runner(nc, NCORES)
    return _RUNNER_CACHE[sig]


def _cols(v, fill=0.0):
    """[NPAD]-padded per-node vector -> [128, GRP] column layout."""
    out = np.full(NPAD, fill, np.float32)
    out[: len(v)] = v
    return out.reshape(GRP, 128).T.copy()


def prepare(x, src, dst, graph_ids, W0, b0, W1, b1, W2, b2, Wc, bc):
    """Host prep: edge tables, x quantization, per-core in_maps + runner."""
    x = np.asarray(x, np.float32)
    graph_ids = np.asarray(graph_ids, np.int64)
    idx_w, slot_cols, norm, CA, CB, choff, NCH, TOT = _prep_edges(
        np.asarray(src), np.asarray(dst))

    sig = (NCH, TOT, CA.tobytes(), CB.tobytes(),
           os.environ.get("KSTAGE", "full"), os.environ.get("KSHARED", "1"),
           os.environ.get("KSPKT", "0"))
    runner = _get_runner(sig, CA, CB, choff, NCH, TOT)

    # int5 quantization of x with per-node scales, 3 values per uint16
    absmax = np.maximum(np.abs(x).max(axis=1), 1e-6)
    scale = (absmax / 15.5).astype(np.float32)
    v5 = np.clip(np.rint(x / scale[:, None] + 15.5), 0, 31).astype(np.int32)
    XW = (F + 2) // 3
    vp = np.zeros((x.shape[0], XW * 3), np.int32)
    vp[:, :F] = v5
    xpk = (vp.reshape(-1, XW, 3) * np.array([1, 32, 1024])).sum(axis=2).astype(np.int16)

    bs = [np.asarray(b0, np.float32), np.asarray(b1, np.float32),
          np.asarray(b2, np.float32)]
    b_cols = np.stack(bs, axis=1).astype(np.float32)            # [128, 3]
    bc_rep = np.tile(np.asarray(bc, np.float32)[None, :], (128, 1))
    wc_pad = np.zeros((F, F), np.float32)
    wc_pad[:, :CLASSES] = np.asarray(Wc, np.float32)
    wpack = np.concatenate([np.asarray(W0, np.float32), np.asarray(W1, np.float32),
                            np.asarray(W2, np.float32), wc_pad],
                           axis=0).astype(np.float16)           # [1280, 128]
    WSH = wpack.shape[0] // NCORES
    in_maps = []
    for c in range(NCORES):
        sl = slice(c * PER, (c + 1) * PER)
        xp_loc = np.zeros((NPAD, XW), np.int16)
        xp_loc[:PER] = xpk[sl]
        in_maps.append(dict(
            xp=xp_loc,
            xs_cols=_cols(scale[sl]).astype(np.float16),
            norm_cols=_cols(norm[sl]).astype(np.float16),
            idx_w=idx_w[c],
            slot_cols=slot_cols[c],
            gslot=_cols(graph_ids[sl].astype(np.float32), -1.0).astype(np.int8),
            wsh=wpack[c * WSH : (c + 1) * WSH],
            b_cols=b_cols,
            bc_rep=bc_rep,
        ))
    return runner, in_maps


def kernel(x, src, dst, graph_ids, W0, b0, W1, b1, W2, b2, Wc, bc, **_):
    runner, in_maps = prepare(x, src, dst, graph_ids, W0, b0, W1, b1, W2, b2,
                              Wc, bc)
    res = runner(in_maps)
    return np.asarray(res[0]["out"], np.float32)
